# revision 1
# baseline (speedup 1.0000x reference)
"""ContactMapHead Trainium2 kernel (v9: best measured configuration).

Reference computation (per batch b):
    h = relu(X @ W^T + pb)            # [S, DP]
    scores = (h @ h^T) * cw + cb      # [S, S]  -- symmetric!

Sharding over 8 NeuronCores: core c handles batch b = c//2 with roll
offset off = (c%2)*1024 applied to X on the host. Each core computes
hT = relu(W @ XT + pb) for its full (rolled) batch, then emits the
circulant band of the symmetric score map: local tile rows i_t in 0..7
(tiles of 128), local cols j_t in i_t..i_t+8 (9 tiles of 128). Across
the two cores of a batch pair plus host-side transpose mirroring this
covers all 16x16 global tiles exactly.

Design (timeline facts from per-core ntff traces; 66.5us -> 46.2us):
- Host transposes X/W into the exact [p, k, s] PE layout and casts to
  bf16 (tolerance 2e-2, bf16 lands at 4.1e-3): eliminates all 144 PE
  transposes (~45% of Tensor busy) + their PSUM copy-backs and halves
  DMA bytes. Output band is bf16, upconverted on the host.
- Input streams over BOTH HWDGE rings (one ring alone ~220 GB/s, both
  together ~330 = per-core HBM cap); 256-col chunks alternate rings
  and complete in index order, feeding the projection as they land.
- wt + tiny constants ride the gpsimd SWDGE ring, keeping the two
  HWDGE rings clear for x chunks (a tiny DMA costs ~1us of ring time).
- ~9 f32 warm-up matmuls on an on-chip identity trip the HAM activity
  monitor while input streams, so real matmuls run at 2.4 GHz
  (cold = 1.2 GHz, ~3.4us window).
- Projection: two 256-col openers (start as soon as ~0.75 MiB landed)
  then 512-wide groups (LDWEIGHTS fully hidden: 213ns/MM measured);
  relu pt0 on ScalarE / pt1 on VectorE (different PSUM banks ->
  parallel). Band rows dovetail between projection groups as their
  rhs span (cols < i*128+1152) completes; 6 band PSUM banks + one
  output buffer per row so copies never stall the PE.
- Warm PE stream floor: 32x110 + 48x213 (proj) + 48x160 (band)
  ~= 21.4us; input (14.2us) and output drain hide underneath it.
"""

import numpy as np
import ml_dtypes

from concourse import bacc, masks, mybir, tile

BF = ml_dtypes.bfloat16

P = 128
B, S, D = 4, 2048, 1024
DP = 256  # projection dim
NCORES = 8
KT = D // P  # 8 k-tiles over D
PT = DP // P  # 2 p-tiles over DP
CHK = 256  # input DMA chunk width (s columns)
NCH = S // CHK  # 8 chunks
NROW = 8  # local band rows (tiles of 128) per core
BANDW = 9 * P  # 1152 band columns per row
SEG = BANDW // 3  # 384-col band chunks
NWARM = 9

f32 = mybir.dt.float32
bf16 = mybir.dt.bfloat16


def _build_nc():
    nc = bacc.Bacc()
    xt = nc.declare_dram_parameter("xt", [P, KT, S], bf16, isOutput=False)
    wt = nc.declare_dram_parameter("wt", [P, KT, DP], bf16, isOutput=False)
    pb = nc.declare_dram_parameter("pb", [DP], f32, isOutput=False)
    cwb = nc.declare_dram_parameter("cwb", [2], f32, isOutput=False)
    out = nc.declare_dram_parameter("out", [NROW, P, BANDW], bf16, isOutput=True)

    with tile.TileContext(nc) as tc:
        _body(nc, tc, xt, wt, pb, cwb, out)
    nc.compile()
    return nc


def _body(nc, tc, xt, wt, pb, cwb, out):
    mult = mybir.AluOpType.mult
    add = mybir.AluOpType.add
    Relu = mybir.ActivationFunctionType.Relu
    Ident = mybir.ActivationFunctionType.Identity

    with (
        tc.tile_pool(name="const", bufs=1) as cpool,
        tc.tile_pool(name="orow", bufs=NROW) as opool,
        tc.tile_pool(name="pj", bufs=2, space="PSUM") as pj,
        tc.tile_pool(name="pw", bufs=6, space="PSUM") as pw,
    ):
        # ---- PE warm-up: f32 matmuls on an on-chip identity (no DMA
        # dependency). They run while the input streams in and trip the
        # HAM activity monitor so real work starts at 2.4 GHz. The psum
        # comes from the band pool and is recycled long before row 0.
        ident = cpool.tile([P, P], f32, tag="ident")
        masks.make_identity(nc, ident[:])
        wps = pw.tile([P, SEG], f32, tag="pw", name="warm")
        for _ in range(NWARM):
            nc.tensor.matmul(wps[:, 0:P], ident[:], ident[:], start=True, stop=True)

        # ---- wt + constants on the gpsimd SWDGE ring: a third DMA ring
        # whose issue cost lands on the otherwise-idle GpSimd engine, so
        # both HWDGE rings belong entirely to the x chunks.
        wt_t = cpool.tile([P, KT, DP], bf16, tag="wt_t")
        nc.gpsimd.dma_start(wt_t[:], wt.ap()[:])

        pb_t = cpool.tile([P, PT], f32, tag="pb_t")
        nc.gpsimd.dma_start(pb_t[:], pb.ap().rearrange("(t p) -> p t", p=P))

        cwb_t = cpool.tile([P, 2], f32, tag="cwb_t")
        nc.gpsimd.dma_start(cwb_t[:], cwb.ap().partition_broadcast(P))

        # ---- x chunks alternate sync/scalar so both HWDGE rings stream
        # (one ring ~220 GB/s, both ~330 = HBM cap) and chunks complete
        # in index order with ~1us spacing.
        xtile = cpool.tile([P, KT, S], bf16, tag="xtile")
        xv = xt.ap()
        for ch in range(NCH):
            c0 = ch * CHK
            eng = nc.sync if ch % 2 == 0 else nc.scalar
            eng.dma_start(xtile[:, :, c0 : c0 + CHK], xv[:, :, c0 : c0 + CHK])

        # hT for the whole local map; relu writes per (pt, chunk) slices
        ht = cpool.tile([P, PT, S], bf16, tag="ht")

        def project(c0, c1):
            w = c1 - c0
            for pt in range(PT):
                pjs = pj.tile([P, 512], f32, tag="pj", name="pj")
                for k in range(KT):
                    nc.tensor.matmul(
                        pjs[:, 0:w],
                        wt_t[:, k, pt * P : (pt + 1) * P],
                        xtile[:, k, c0:c1],
                        start=(k == 0),
                        stop=(k == KT - 1),
                    )
                # pt0 -> ScalarE, pt1 -> VectorE so the two relu+bias
                # passes run in parallel.
                if pt == 0:
                    nc.scalar.activation(
                        ht[:, pt, c0:c1],
                        pjs[:, 0:w],
                        Relu,
                        bias=pb_t[:, pt : pt + 1],
                    )
                else:
                    nc.vector.tensor_scalar(
                        ht[:, pt, c0:c1],
                        pjs[:, 0:w],
                        pb_t[:, pt : pt + 1],
                        0.0,
                        add,
                        mybir.AluOpType.max,
                    )

        def emit_pair_row(i_t):
            """Band row i_t: out[i_t] = cw * hT_i^T @ hT[band cols] + cb."""
            base = i_t * P
            psums = []
            for pt in range(PT):
                for si in range(3):
                    if pt == 0:
                        psums.append(pw.tile([P, SEG], f32, tag="pw", name="pw"))
                    c0 = base + si * SEG
                    nc.tensor.matmul(
                        psums[si][:],
                        ht[:, pt, base : base + P],
                        ht[:, pt, c0 : c0 + SEG],
                        start=(pt == 0),
                        stop=(pt == PT - 1),
                    )
            orow = opool.tile([P, BANDW], bf16, tag="orow", name="orow")
            tail = i_t >= NROW - 2
            for si in range(3):
                dst = orow[:, si * SEG : (si + 1) * SEG]
                if (i_t * 3 + si) % 2 == 0:
                    nc.vector.tensor_scalar(
                        dst, psums[si][:], cwb_t[:, 0:1], cwb_t[:, 1:2], mult, add
                    )
                else:
                    nc.scalar.activation(
                        dst, psums[si][:], Ident,
                        bias=cwb_t[:, 1:2], scale=cwb_t[:, 0:1],
                    )
                if tail:
                    # last rows: drain per segment so the final DMA is small
                    eng = nc.sync if (i_t + si) % 2 == 0 else nc.gpsimd
                    eng.dma_start(
                        out.ap()[i_t][:, si * SEG : (si + 1) * SEG], dst
                    )
            if not tail:
                # out-DMAs issue from sync/gpsimd: a DMA issue costs ~600ns
                # on the issuing engine and ScalarE is busy with copies
                eng = nc.sync if i_t % 2 == 0 else nc.gpsimd
                eng.dma_start(out.ap()[i_t], orow[:])

        # dovetail: projection in two 256-col openers (start on the first
        # DMA chunk) then 512-wide groups (LDW fully hidden, 213ns/MM);
        # band rows emitted as soon as the chunk completing their rhs
        # span (cols < i*128+1152) is done.
        project(0, 256)
        project(256, 512)
        project(512, 1024)
        project(1024, 1536)
        for i_t in range(4):
            emit_pair_row(i_t)
        project(1536, 2048)
        for i_t in range(4, NROW):
            emit_pair_row(i_t)


_NC_CACHE = None


def _get_nc():
    global _NC_CACHE
    if _NC_CACHE is None:
        _NC_CACHE = _build_nc()
    return _NC_CACHE


def _pack_pks(mat_T, rows, cols):
    """[rows*P, cols] -> [P, rows, cols] with d = k*P + p split as (k, p)."""
    return np.ascontiguousarray(
        mat_T.reshape(rows, P, cols).transpose(1, 0, 2)
    )


def _make_in_maps(hidden_states, proj_w, proj_b, clf_w, clf_b):
    hs = np.asarray(hidden_states, dtype=np.float32)
    wv = np.asarray(proj_w, dtype=np.float32)
    pbv = np.ascontiguousarray(np.asarray(proj_b, dtype=np.float32).reshape(DP))
    cwbv = np.array(
        [np.asarray(clf_w).reshape(-1)[0], np.asarray(clf_b).reshape(-1)[0]],
        dtype=np.float32,
    )
    wtv = _pack_pks(wv.astype(BF).T, KT, DP)  # [P, KT, DP]
    in_maps = []
    for b in range(B):
        xpks = _pack_pks(hs[b].astype(BF).T, KT, S)  # [P, KT, S]
        xpks_r = np.ascontiguousarray(np.roll(xpks, -S // 2, axis=2))
        for xv_ in (xpks, xpks_r):
            in_maps.append({"xt": xv_, "wt": wtv, "pb": pbv, "cwb": cwbv})
    return in_maps


def _assemble(results):
    scores = np.empty((B, S, S), np.float32)
    for c in range(NCORES):
        b, half = divmod(c, 2)
        o = np.asarray(results[c]["out"], dtype=np.float32)  # [NROW, P, BANDW]
        for i_t in range(NROW):
            gi = i_t + NROW * half
            strip = o[i_t]
            for lj in range(i_t, i_t + 9):
                gj = (lj + NROW * half) % 16
                V = strip[:, (lj - i_t) * P : (lj - i_t + 1) * P]
                scores[b, gi * P : (gi + 1) * P, gj * P : (gj + 1) * P] = V
                if gj != gi:
                    scores[b, gj * P : (gj + 1) * P, gi * P : (gi + 1) * P] = V.T
    return scores


def kernel(hidden_states, proj_w, proj_b, clf_w, clf_b):
    from concourse.bass_utils import run_bass_kernel_spmd

    nc = _get_nc()
    in_maps = _make_in_maps(hidden_states, proj_w, proj_b, clf_w, clf_b)
    res = run_bass_kernel_spmd(nc, in_maps, core_ids=list(range(NCORES)))
    return _assemble(res.results)


def run_traced(hidden_states, proj_w, proj_b, clf_w, clf_b):
    """Like kernel(), but also returns BassKernelResults with trace info."""
    from concourse.bass_utils import run_bass_kernel_spmd

    nc = _get_nc()
    in_maps = _make_in_maps(hidden_states, proj_w, proj_b, clf_w, clf_b)
    res = run_bass_kernel_spmd(
        nc, in_maps, core_ids=list(range(NCORES)), trace=True
    )
    return _assemble(res.results), res

